# revision 31
# baseline (speedup 1.0000x reference)
"""Trainium2 Bass kernel for nn_EquivariantConvolution (gnn_message_passing).

Math (per edge e):
    h  = relu(edge_feats @ W1 + b1)            [E,128]
    rw = (h @ W2 + b2) -> [E, 16, 48]
    fe = f[U]                                  [E,16,3]
    tmp[e,m,k] = sum_d fe[e,m,d] * basis[e,d,k]        (k = r*3+dd, 9)
    out[e,i,dd] = sum_{m,r} rw[e,i,m*3+r] * tmp[e,m,r*3+dd]

v2 "A-order" contraction (same result, fewer DVE passes):
    A[e,i,r,d]  = sum_m rw[e,(i,r,m)] * fe[e,(d,m)]     (contract m=16)
    out[e,i,dd] = sum_{r,d} A[e,i,(r,d)] * basis[e,(dd,r,d)]  (contract 9)

Sharding: edges split across 8 cores (40000 each, padded to 40960);
f + MLP weights replicated. Edge j of a 128-edge tile on partition j%128.

Engines: PE does the two MLP GEMMs (+b2 via ones-matmul into PSUM);
ACT evicts rw PSUM->SBUF fp16; gpsimd drives the f[U] dma_gather; DVE
does all per-edge contraction math in fp16 2x mode (products m-packed,
then a halving add-tree per contraction). Instructions are batched over
KTB=4 tiles to amortize fixed per-instruction costs.

HW-measured dead ends (sim underprices both): gpsimd tensor_tensor has
~1-2us fixed cost per instruction (Q7 software), and small-N
strided-ifmap PE matmuls (identity-weight segment sums) cost ~340ns
each vs 120ns modeled. Both offload paths remain behind KENG/KMSUM
env knobs but default off.
"""
import sys

sys.path.insert(0, "/opt/trn_rl_repo")

import os
import numpy as np
import concourse.bass as bass
import concourse.bacc as bacc
import concourse.mybir as mybir
import concourse.tile as tile
from concourse.bass_utils import run_bass_kernel_spmd
from contextlib import ExitStack

# problem constants (hardcoded per harness contract)
E = 320000
N = 10000
M1 = 16
M2 = 16
D1 = 3
D2 = 3
NREPS = 3
EDGE_DIM = 32
HIDDEN = 128
RW = NREPS * M1 * M2  # 768

NCORES = 8
ES = E // NCORES          # 40000 edges per core
ESP_G = 40960             # gather-side pad (40 groups of 1024 idxs)
NTILES = int(os.environ.get('KNT', '316'))     # compute tiles (316*128=40448)
ESP = NTILES * 128        # compute-side padded edges per core
BLK = int(os.environ.get('KBLK', '32'))        # tiles per (full) block
BLKS = [BLK] * (NTILES // BLK)                 # per-block tile counts
if NTILES % BLK:
    BLKS.append(NTILES % BLK)
TB = int(os.environ.get('KTB', '4'))           # tiles batched per DVE inst
assert all(b % TB == 0 and (b * 128) % 512 == 0 for b in BLKS)

_CACHE = {}

ABL = set(os.environ.get("KABL", "").split(","))  # ablation flags
HOSTG = os.environ.get("KHOSTG", "0") == "1"      # gather f[U] on host
FPAD = 128                # f rows padded to 128 fp16 (256B) for dma_gather

# per-op engine map: v = DVE (vector), g = gpsimd (Pool). For G (one inst
# per tile in the TB batch), a multi-char string assigns per tile index.
_ENG_DEFAULT = "P:v,L1:v,L2:v,L3:v,L4:v,G:vvvv,T1:v,T2:v,T3:v,T4:v"
ENG_MAP = dict(kv.split(":") for kv in
               os.environ.get("KENG", _ENG_DEFAULT).split(","))
L1V = int(os.environ.get("KL1V", "48"))  # ir columns of L1 on DVE; rest gpsimd
MSUM = os.environ.get("KMSUM", "dve")    # m-sum: pe | dve | mix (per-quad)
MIXN = int(os.environ.get("KMIXN", "1"))  # mix: PE-path quads per KMIXD
MIXD = int(os.environ.get("KMIXD", "2"))


def _build(reps=1):
    dt = mybir.dt
    nc = bacc.Bacc("TRN2", target_bir_lowering=False, debug=False,
                   num_devices=NCORES)

    efT_d = nc.dram_tensor("efT", [EDGE_DIM, ESP], dt.float16, kind="ExternalInput").ap()
    basis_d = nc.dram_tensor("basisp", [128, NTILES * 27], dt.float16, kind="ExternalInput").ap()
    if HOSTG:
        fep_d = nc.dram_tensor("fep", [128, NTILES * 48], dt.float16, kind="ExternalInput").ap()
    else:
        uw_d = nc.dram_tensor("uw", [128, ESP_G // 16], dt.int16, kind="ExternalInput").ap()
        fpad_d = nc.dram_tensor("fpad", [N, FPAD], dt.float16, kind="ExternalInput").ap()
        fep_d = None
    w1_d = nc.dram_tensor("w1", [EDGE_DIM, HIDDEN], dt.float16, kind="ExternalInput").ap()
    b1_d = nc.dram_tensor("b1", [HIDDEN, 1], dt.float32, kind="ExternalInput").ap()
    w2_d = nc.dram_tensor("w2r", [HIDDEN, RW], dt.float16, kind="ExternalInput").ap()
    b2_d = nc.dram_tensor("b2r", [1, RW], dt.float16, kind="ExternalInput").ap()
    ones_d = nc.dram_tensor("ones1", [1, 128], dt.float16, kind="ExternalInput").ap()
    ident_d = nc.dram_tensor("ident", [128, 128], dt.float16, kind="ExternalInput").ap()
    out_d = nc.dram_tensor("outp", [128, NTILES * 48], dt.float16, kind="ExternalOutput").ap()

    with tile.TileContext(nc) as tc, ExitStack() as ctx:
        cpool = ctx.enter_context(tc.tile_pool(name="const", bufs=1))
        inpool = ctx.enter_context(tc.tile_pool(name="in", bufs=2))
        hpool = ctx.enter_context(tc.tile_pool(name="h", bufs=2))
        wpool = ctx.enter_context(tc.tile_pool(name="work", bufs=2))
        spool = ctx.enter_context(tc.tile_pool(name="small", bufs=2))
        opool = ctx.enter_context(tc.tile_pool(name="out", bufs=2))
        pps = ctx.enter_context(tc.tile_pool(name="psA", bufs=2, space="PSUM"))
        ppr = ctx.enter_context(tc.tile_pool(name="psB", bufs=2, space="PSUM"))
        ppa = ctx.enter_context(tc.tile_pool(name="psC", bufs=2, space="PSUM"))

        w1_sb = cpool.tile([EDGE_DIM, HIDDEN], dt.float16)
        nc.sync.dma_start(w1_sb[:], w1_d[:])
        b1_sb = cpool.tile([HIDDEN, 1], dt.float32)
        nc.sync.dma_start(b1_sb[:], b1_d[:])
        w2_sb = cpool.tile([HIDDEN, RW], dt.float16)
        nc.sync.dma_start(w2_sb[:], w2_d[:])
        b2_sb = cpool.tile([1, RW], dt.float16)
        nc.sync.dma_start(b2_sb[:], b2_d[:])
        ones_sb = cpool.tile([1, 128], dt.float16)
        nc.sync.dma_start(ones_sb[:], ones_d[:])
        ident_sb = cpool.tile([128, 128], dt.float16)
        nc.sync.dma_start(ident_sb[:], ident_d[:])
        if HOSTG:
            uw_sb = fpad_dd = None
        else:
            uw_sb = cpool.tile([128, ESP_G // 16], dt.int16)
            nc.sync.dma_start(uw_sb[:], uw_d[:])
            fpad_dd = fpad_d

        def body():
            _body(nc, tc, dt, inpool, hpool, wpool, spool, opool, pps, ppr,
                  ppa, efT_d, basis_d, fep_d, uw_sb, fpad_dd,
                  w1_sb, b1_sb, w2_sb, b2_sb, ones_sb, ident_sb, out_d)

        if reps == 1:
            body()
        else:
            with tc.For_i(0, reps, 1):
                body()

    nc.compile()
    return nc


def _body(nc, tc, dt, inpool, hpool, wpool, spool, opool, pps, ppr,
          ppa, efT_d, basis_d, fep_d, uw_sb, fpad_d,
          w1_sb, b1_sb, w2_sb, b2_sb, ones_sb, ident_sb, out_d):
    RD = NREPS * D1   # 9
    FW = 48 if HOSTG else FPAD
    mul = mybir.AluOpType.mult
    add = mybir.AluOpType.add
    def _e(ch):
        return nc.vector if ch == "v" else nc.gpsimd
    eng = {k: _e(v[0]) for k, v in ENG_MAP.items()}
    geng = [_e(ENG_MAP["G"][u % len(ENG_MAP["G"])]) for u in range(TB)]
    with nc.allow_low_precision(reason="fp16 pipeline; abs gate 2e-2"):
        t0g = 0  # first tile of this block
        for b, blk in enumerate(BLKS):
            eblk = blk * 128
            e0 = t0g * 128
            # block loads
            efT_sb = inpool.tile([EDGE_DIM, eblk], dt.float16, tag="efT")
            nc.sync.dma_start(efT_sb[:], efT_d[:, e0:e0 + eblk])
            basis_sb = inpool.tile([128, blk, 27], dt.float16, tag="basis")
            nc.sync.dma_start(basis_sb[:].rearrange("p b k -> p (b k)"),
                              basis_d[:, t0g * 27:(t0g + blk) * 27])
            fe_sb = inpool.tile([128, blk, FW], dt.float16, tag="fe")
            if HOSTG:
                nc.sync.dma_start(
                    fe_sb[:].rearrange("p b k -> p (b k)"),
                    fep_d[:, t0g * 48:(t0g + blk) * 48])
            else:
                for g in range((eblk + 1023) // 1024):
                    nidx = min(1024, eblk - g * 1024)
                    i0 = e0 // 16 + g * 64
                    nc.gpsimd.dma_gather(
                        fe_sb[:, g * 8:g * 8 + nidx // 128, :], fpad_d[:],
                        uw_sb[:, i0:i0 + nidx // 16],
                        num_idxs=nidx, num_idxs_reg=nidx, elem_size=FPAD,
                    )

            # h.T = relu(W1.T @ efT + b1): [128h, eblk] fp16
            hT_sb = hpool.tile([HIDDEN, eblk], dt.float16, tag="hT")
            for q in range(eblk // 512 if "nomlp" not in ABL else 0):
                hT_ps = pps.tile([HIDDEN, 512], dt.float32, tag="hTps")
                nc.tensor.matmul(hT_ps[:], w1_sb[:],
                                 efT_sb[:, q * 512:(q + 1) * 512],
                                 start=True, stop=True)
                nc.scalar.activation(hT_sb[:, q * 512:(q + 1) * 512], hT_ps[:],
                                     mybir.ActivationFunctionType.Relu,
                                     bias=b1_sb[:], scale=1.0)

            out_sb = opool.tile([128, blk, 48], dt.float16, tag="outsb")

            for tq in range(blk // TB):
                # rw for TB tiles: PE matmul + b2 ones-matmul, ACT eviction
                rw_sb = wpool.tile([128, TB, RW], dt.float16, tag="rwsb")
                for u in range(TB):
                    t = tq * TB + u
                    rw_ps = ppr.tile([128, RW], dt.float32, tag="rwps")
                    hT_c = hT_sb[:, t * 128:(t + 1) * 128]
                    if "nomlp" not in ABL:
                        nc.tensor.matmul(rw_ps[:, 0:512], hT_c, w2_sb[:, 0:512],
                                         start=True, stop=False)
                        nc.tensor.matmul(rw_ps[:, 0:512], ones_sb[:],
                                         b2_sb[:, 0:512], start=False, stop=True)
                        nc.tensor.matmul(rw_ps[:, 512:RW], hT_c, w2_sb[:, 512:RW],
                                         start=True, stop=False)
                        nc.tensor.matmul(rw_ps[:, 512:RW], ones_sb[:],
                                         b2_sb[:, 512:RW], start=False, stop=True)
                    nc.scalar.activation(rw_sb[:, u, :], rw_ps[:],
                                         mybir.ActivationFunctionType.Copy,
                                         bias=0.0, scale=1.0)

                if "noc" in ABL:
                    continue
                # products1: P[p,t,(i r),d,m] = rw[p,t,(i r),m] * fe[p,t,d,m]
                rw_b = rw_sb[:].rearrange(
                    "p t (ir m) -> p t ir m", ir=48, m=M1
                ).unsqueeze(3).broadcast_to([128, TB, 48, D1, M1])
                fe_b = fe_sb[:, tq * TB:(tq + 1) * TB, 0:48].rearrange(
                    "p t (d m) -> p t d m", d=D1, m=M1
                ).unsqueeze(2).broadcast_to([128, TB, 48, D1, M1])
                P = wpool.tile([128, TB, 48, D1, M1], dt.float16, tag="P")
                eng["P"].tensor_tensor(P[:], rw_b, fe_b, mul)
                A = spool.tile([128, TB, 48, D1], dt.float16, tag="A")
                qidx = t0g // TB + tq
                use_pe = MSUM == "pe" or (MSUM == "mix" and
                                          qidx % MIXD < MIXN)
                if use_pe:
                    # m-sum on PE: 16 identity-weight matmuls accumulate the
                    # m-slices of P into PSUM (half-quad: N=288 <= 512 fp32)
                    for h in range(TB // 2):
                        A_ps = ppa.tile([128, 2, 48, D1], dt.float32, tag="Aps")
                        for j in range(M1):
                            nc.tensor.matmul(A_ps[:], ident_sb[:],
                                             P[:, h * 2:(h + 1) * 2, :, :, j],
                                             start=(j == 0), stop=(j == M1 - 1))
                        nc.scalar.activation(A[:, h * 2:(h + 1) * 2], A_ps[:],
                                             mybir.ActivationFunctionType.Copy,
                                             bias=0.0, scale=1.0)
                else:
                    # tree over m on DVE/gpsimd: 16 -> 8 -> 4 -> 2 -> 1
                    P8 = wpool.tile([128, TB, 48, D1, 8], dt.float16, tag="P8")
                    if L1V >= 48:
                        eng["L1"].tensor_tensor(P8[:], P[:, :, :, :, 0:8],
                                                P[:, :, :, :, 8:16], add)
                    else:
                        nc.vector.tensor_tensor(P8[:, :, 0:L1V], P[:, :, 0:L1V, :, 0:8],
                                                P[:, :, 0:L1V, :, 8:16], add)
                        nc.gpsimd.tensor_tensor(P8[:, :, L1V:48], P[:, :, L1V:48, :, 0:8],
                                                P[:, :, L1V:48, :, 8:16], add)
                    P4 = spool.tile([128, TB, 48, D1, 4], dt.float16, tag="P4")
                    eng["L2"].tensor_tensor(P4[:], P8[:, :, :, :, 0:4],
                                            P8[:, :, :, :, 4:8], add)
                    P2 = spool.tile([128, TB, 48, D1, 2], dt.float16, tag="P2")
                    eng["L3"].tensor_tensor(P2[:], P4[:, :, :, :, 0:2],
                                            P4[:, :, :, :, 2:4], add)
                    eng["L4"].tensor_tensor(A[:], P2[:, :, :, :, 0],
                                            P2[:, :, :, :, 1], add)

                # products2 per tile: G[p,i,dd,(r d)] = A * basis
                Gq = spool.tile([128, TB, 48, RD], dt.float16, tag="Gq")
                for u in range(TB):
                    t = tq * TB + u
                    A_b = A[:, u].rearrange(
                        "p (i r) d -> p i (r d)", i=M2, r=NREPS
                    ).unsqueeze(2).broadcast_to([128, M2, D2, RD])
                    ba_b = basis_sb[:, t].rearrange(
                        "p (dd rd) -> p dd rd", dd=D2, rd=RD
                    ).unsqueeze(1).broadcast_to([128, M2, D2, RD])
                    g_t = Gq[:, u].rearrange(
                        "p (i dd) rd -> p i dd rd", i=M2, dd=D2)
                    geng[u].tensor_tensor(g_t, A_b, ba_b, mul)
                # tree over (r d): 9 = 8 + 1
                T1 = spool.tile([128, TB, 48, 4], dt.float16, tag="T1")
                eng["T1"].tensor_tensor(T1[:], Gq[:, :, :, 0:4],
                                        Gq[:, :, :, 4:8], add)
                T2 = spool.tile([128, TB, 48, 2], dt.float16, tag="T2")
                eng["T2"].tensor_tensor(T2[:], T1[:, :, :, 0:2],
                                        T1[:, :, :, 2:4], add)
                T3 = spool.tile([128, TB, 48], dt.float16, tag="T3")
                eng["T3"].tensor_tensor(T3[:], T2[:, :, :, 0], T2[:, :, :, 1],
                                        add)
                eng["T4"].tensor_tensor(out_sb[:, tq * TB:(tq + 1) * TB, :],
                                        T3[:], Gq[:, :, :, 8], add)

            if "noc" not in ABL:
                nc.sync.dma_start(out_d[:, t0g * 48:(t0g + blk) * 48],
                                  out_sb[:].rearrange("p b k -> p (b k)"))
            t0g += blk


def _get_nc(reps=1):
    key = ("nc", reps)
    if key not in _CACHE:
        _CACHE[key] = _build(reps)
    return _CACHE[key]


def _prep_core(U_c, basis_c, ef_c, f, W1, b1, W2, b2):
    """Build one core's input map (host-side layout/swizzle)."""
    ne = U_c.shape[0]
    U_p = np.concatenate([np.asarray(U_c, np.int64),
                          np.zeros(ESP_G - ne, np.int64)])
    basis_p = np.concatenate(
        [np.asarray(basis_c, np.float32).reshape(-1, D1, NREPS * D2),
         np.zeros((ESP - ne, D1, NREPS * D2), np.float32)], axis=0)
    ef_p = np.concatenate(
        [np.asarray(ef_c, np.float32),
         np.zeros((ESP - ne, EDGE_DIM), np.float32)], axis=0)

    efT = np.ascontiguousarray(ef_p.T).astype(np.float16)                # [32, ESP]
    # basis edge layout (dd, r, d): idx = dd*9 + r*3 + d from in [d, r*3+dd]
    bp = basis_p.reshape(ESP, D1, NREPS, D2)          # [e, d, r, dd]
    bp = bp.transpose(0, 3, 2, 1)                     # [e, dd, r, d]
    bp = bp.reshape(ESP, 27)
    basisp = np.ascontiguousarray(
        bp.reshape(NTILES, 128, 27).transpose(1, 0, 2).reshape(128, NTILES * 27)
    ).astype(np.float16)
    # source-node features, d-major per node: [N, (d, m)]
    f48 = np.ascontiguousarray(
        np.asarray(f, np.float32).transpose(0, 2, 1).reshape(N, D1 * M1)
    ).astype(np.float16)
    if HOSTG:
        fe_all = f48[U_p[:ESP]]                                          # [ESP, 48]
        fmaps = {"fep": np.ascontiguousarray(
            fe_all.reshape(NTILES, 128, 48).transpose(1, 0, 2)
            .reshape(128, NTILES * 48))}
    else:
        uw16 = U_p.astype(np.int16).reshape(ESP_G // 1024, 64, 16).transpose(2, 0, 1)
        fpad = np.zeros((N, FPAD), np.float16)
        fpad[:, :M1 * D1] = f48
        fmaps = {
            "uw": np.ascontiguousarray(
                np.tile(uw16.reshape(16, ESP_G // 16), (8, 1))),
            "fpad": fpad,
        }
    # W2/b2 column reorder: (i, m, r) -> (i, r, m)
    w2r = np.asarray(W2, np.float32).reshape(HIDDEN, M2, M1, NREPS)
    w2r = np.ascontiguousarray(w2r.transpose(0, 1, 3, 2).reshape(HIDDEN, RW))
    b2r = np.asarray(b2, np.float32).reshape(M2, M1, NREPS)
    b2r = np.ascontiguousarray(b2r.transpose(0, 2, 1).reshape(1, RW))
    return {
        "efT": efT,
        "basisp": basisp,
        **fmaps,
        "w1": np.asarray(W1, np.float32).astype(np.float16),
        "b1": np.asarray(b1, np.float32).reshape(HIDDEN, 1),
        "w2r": w2r.astype(np.float16),
        "b2r": b2r.astype(np.float16),
        "ones1": np.ones((1, 128), np.float16),
        "ident": np.eye(128, dtype=np.float16),
    }


def kernel(U, basis, edge_feats, f, W1, b1, W2, b2):
    U = np.asarray(U)
    basis = np.asarray(basis, np.float32)
    edge_feats = np.asarray(edge_feats, np.float32)
    nc = _get_nc()
    in_maps = []
    for c in range(NCORES):
        sl = slice(c * ES, (c + 1) * ES)
        in_maps.append(_prep_core(U[sl], basis[sl], edge_feats[sl],
                                  f, W1, b1, W2, b2))
    res = run_bass_kernel_spmd(nc, in_maps, core_ids=list(range(NCORES)))
    outs = []
    for c in range(NCORES):
        op = res.results[c]["outp"]                                   # [128, NTILES*48] fp16
        o = op.astype(np.float32).reshape(128, NTILES, 48)
        o = o.transpose(1, 0, 2).reshape(ESP, 48)
        outs.append(o[:ES])
    return np.concatenate(outs, axis=0).reshape(E, M2, D2).astype(np.float32)


if __name__ == "__main__":
    rng = np.random.default_rng(0)
    inputs = {
        "U": rng.integers(0, N, size=E),
        "basis": rng.standard_normal((E, D1, NREPS * D2), dtype=np.float32),
        "edge_feats": rng.standard_normal((E, EDGE_DIM), dtype=np.float32),
        "f": rng.standard_normal((N, M1, D1), dtype=np.float32),
        "W1": (rng.standard_normal((EDGE_DIM, HIDDEN), dtype=np.float32) / np.sqrt(EDGE_DIM)),
        "b1": rng.standard_normal(HIDDEN, dtype=np.float32) * 0.02,
        "W2": (rng.standard_normal((HIDDEN, RW), dtype=np.float32) / np.sqrt(HIDDEN)),
        "b2": rng.standard_normal(RW, dtype=np.float32) * 0.02,
    }
    out = kernel(**inputs)
    print(out.shape, out.dtype)


# revision 32
# speedup vs baseline: 1.2291x; 1.2291x over previous
"""Trainium2 Bass kernel for nn_EquivariantConvolution (gnn_message_passing).

Math (per edge e):
    h  = relu(edge_feats @ W1 + b1)            [E,128]
    rw = (h @ W2 + b2) -> [E, 16, 48]
    fe = f[U]                                  [E,16,3]
    tmp[e,m,k] = sum_d fe[e,m,d] * basis[e,d,k]        (k = r*3+dd, 9)
    out[e,i,dd] = sum_{m,r} rw[e,i,m*3+r] * tmp[e,m,r*3+dd]

v2 "A-order" contraction (same result, fewer DVE passes):
    A[e,i,r,d]  = sum_m rw[e,(i,r,m)] * fe[e,(d,m)]     (contract m=16)
    out[e,i,dd] = sum_{r,d} A[e,i,(r,d)] * basis[e,(dd,r,d)]  (contract 9)

Sharding: edges split across 8 cores (40000 each, padded to 40960);
f + MLP weights replicated. Edge j of a 128-edge tile on partition j%128.

Engines: PE does the two MLP GEMMs (+b2 via ones-matmul into PSUM);
ACT evicts rw PSUM->SBUF fp16; gpsimd drives the f[U] dma_gather; DVE
does all per-edge contraction math in fp16 2x mode (products m-packed,
then a halving add-tree per contraction). Instructions are batched over
KTB=4 tiles to amortize fixed per-instruction costs.

HW-measured dead ends (sim underprices both): gpsimd tensor_tensor has
~1-2us fixed cost per instruction (Q7 software), and small-N
strided-ifmap PE matmuls (identity-weight segment sums) cost ~340ns
each vs 120ns modeled. Both offload paths remain behind KENG/KMSUM
env knobs but default off.
"""
import sys

sys.path.insert(0, "/opt/trn_rl_repo")

import os
import numpy as np
import concourse.bass as bass
import concourse.bacc as bacc
import concourse.mybir as mybir
import concourse.tile as tile
from concourse.bass_utils import run_bass_kernel_spmd
from contextlib import ExitStack

# problem constants (hardcoded per harness contract)
E = 320000
N = 10000
M1 = 16
M2 = 16
D1 = 3
D2 = 3
NREPS = 3
EDGE_DIM = 32
HIDDEN = 128
RW = NREPS * M1 * M2  # 768

NCORES = 8
ES = E // NCORES          # 40000 edges per core
ESP_G = 40960             # gather-side pad (40 groups of 1024 idxs)
NTILES = int(os.environ.get('KNT', '320'))     # compute tiles (all 1024-aligned)
ESP = NTILES * 128        # compute-side padded edges per core
BLK = int(os.environ.get('KBLK', '32'))        # tiles per (full) block
BLKS = [BLK] * (NTILES // BLK)                 # per-block tile counts
if NTILES % BLK:
    BLKS.append(NTILES % BLK)
TB = int(os.environ.get('KTB', '4'))           # tiles batched per DVE inst
assert all(b % TB == 0 and (b * 128) % 512 == 0 for b in BLKS)

_CACHE = {}

ABL = set(os.environ.get("KABL", "").split(","))  # ablation flags
HOSTG = os.environ.get("KHOSTG", "0") == "1"      # gather f[U] on host
FPAD = 128                # f rows padded to 128 fp16 (256B) for dma_gather

# per-op engine map: v = DVE (vector), g = gpsimd (Pool). For G (one inst
# per tile in the TB batch), a multi-char string assigns per tile index.
_ENG_DEFAULT = "P:v,L1:v,L2:v,L3:v,L4:v,G:vvvv,T1:v,T2:v,T3:v,T4:v"
ENG_MAP = dict(kv.split(":") for kv in
               os.environ.get("KENG", _ENG_DEFAULT).split(","))
L1V = int(os.environ.get("KL1V", "48"))  # ir columns of L1 on DVE; rest gpsimd
MSUM = os.environ.get("KMSUM", "dve")    # m-sum: pe | dve | mix (per-quad)
MIXN = int(os.environ.get("KMIXN", "1"))  # mix: PE-path quads per KMIXD
MIXD = int(os.environ.get("KMIXD", "2"))


def _build(reps=1):
    dt = mybir.dt
    nc = bacc.Bacc("TRN2", target_bir_lowering=False, debug=False,
                   num_devices=NCORES)

    efT_d = nc.dram_tensor("efT", [EDGE_DIM, ESP], dt.float16, kind="ExternalInput").ap()
    basis_d = nc.dram_tensor("basisp", [128, NTILES * 27], dt.float16, kind="ExternalInput").ap()
    if HOSTG:
        fep_d = nc.dram_tensor("fep", [128, NTILES * 48], dt.float16, kind="ExternalInput").ap()
    else:
        uw_d = nc.dram_tensor("uw", [128, ESP_G // 16], dt.int16, kind="ExternalInput").ap()
        fpad_d = nc.dram_tensor("fpad", [N, FPAD], dt.float16, kind="ExternalInput").ap()
        fep_d = None
    w1_d = nc.dram_tensor("w1", [EDGE_DIM, HIDDEN], dt.float16, kind="ExternalInput").ap()
    b1_d = nc.dram_tensor("b1", [HIDDEN, 1], dt.float32, kind="ExternalInput").ap()
    w2_d = nc.dram_tensor("w2r", [HIDDEN, RW], dt.float16, kind="ExternalInput").ap()
    b2_d = nc.dram_tensor("b2r", [1, RW], dt.float16, kind="ExternalInput").ap()
    ones_d = nc.dram_tensor("ones1", [1, 128], dt.float16, kind="ExternalInput").ap()
    ident_d = nc.dram_tensor("ident", [128, 128], dt.float16, kind="ExternalInput").ap()
    out_d = nc.dram_tensor("outp", [128, NTILES * 48], dt.float16, kind="ExternalOutput").ap()

    with tile.TileContext(nc) as tc, ExitStack() as ctx:
        cpool = ctx.enter_context(tc.tile_pool(name="const", bufs=1))
        inpool = ctx.enter_context(tc.tile_pool(name="in", bufs=2))
        hpool = ctx.enter_context(tc.tile_pool(name="h", bufs=2))
        wpool = ctx.enter_context(tc.tile_pool(name="work", bufs=2))
        spool = ctx.enter_context(tc.tile_pool(name="small", bufs=2))
        opool = ctx.enter_context(tc.tile_pool(name="out", bufs=2))
        pps = ctx.enter_context(tc.tile_pool(name="psA", bufs=2, space="PSUM"))
        ppr = ctx.enter_context(tc.tile_pool(name="psB", bufs=2, space="PSUM"))
        ppa = ctx.enter_context(tc.tile_pool(name="psC", bufs=2, space="PSUM"))

        w1_sb = cpool.tile([EDGE_DIM, HIDDEN], dt.float16)
        nc.sync.dma_start(w1_sb[:], w1_d[:])
        b1_sb = cpool.tile([HIDDEN, 1], dt.float32)
        nc.sync.dma_start(b1_sb[:], b1_d[:])
        w2_sb = cpool.tile([HIDDEN, RW], dt.float16)
        nc.sync.dma_start(w2_sb[:], w2_d[:])
        b2_sb = cpool.tile([1, RW], dt.float16)
        nc.sync.dma_start(b2_sb[:], b2_d[:])
        ones_sb = cpool.tile([1, 128], dt.float16)
        nc.sync.dma_start(ones_sb[:], ones_d[:])
        ident_sb = cpool.tile([128, 128], dt.float16)
        nc.sync.dma_start(ident_sb[:], ident_d[:])
        if HOSTG:
            uw_sb = fpad_dd = None
        else:
            uw_sb = cpool.tile([128, ESP_G // 16], dt.int16)
            nc.sync.dma_start(uw_sb[:], uw_d[:])
            fpad_dd = fpad_d

        def body():
            _body(nc, tc, dt, inpool, hpool, wpool, spool, opool, pps, ppr,
                  ppa, efT_d, basis_d, fep_d, uw_sb, fpad_dd,
                  w1_sb, b1_sb, w2_sb, b2_sb, ones_sb, ident_sb, out_d)

        if reps == 1:
            body()
        else:
            with tc.For_i(0, reps, 1):
                body()

    nc.compile()
    return nc


def _body(nc, tc, dt, inpool, hpool, wpool, spool, opool, pps, ppr,
          ppa, efT_d, basis_d, fep_d, uw_sb, fpad_d,
          w1_sb, b1_sb, w2_sb, b2_sb, ones_sb, ident_sb, out_d):
    RD = NREPS * D1   # 9
    FW = 48 if HOSTG else FPAD
    mul = mybir.AluOpType.mult
    add = mybir.AluOpType.add
    def _e(ch):
        return nc.vector if ch == "v" else nc.gpsimd
    eng = {k: _e(v[0]) for k, v in ENG_MAP.items()}
    geng = [_e(ENG_MAP["G"][u % len(ENG_MAP["G"])]) for u in range(TB)]
    with nc.allow_low_precision(reason="fp16 pipeline; abs gate 2e-2"):
        t0g = 0  # first tile of this block
        for b, blk in enumerate(BLKS):
            eblk = blk * 128
            e0 = t0g * 128
            # block loads
            efT_sb = inpool.tile([EDGE_DIM, eblk], dt.float16, tag="efT")
            nc.sync.dma_start(efT_sb[:], efT_d[:, e0:e0 + eblk])
            basis_sb = inpool.tile([128, blk, 27], dt.float16, tag="basis")
            nc.sync.dma_start(basis_sb[:].rearrange("p b k -> p (b k)"),
                              basis_d[:, t0g * 27:(t0g + blk) * 27])
            fe_sb = inpool.tile([128, blk, FW], dt.float16, tag="fe")
            if HOSTG:
                nc.sync.dma_start(
                    fe_sb[:].rearrange("p b k -> p (b k)"),
                    fep_d[:, t0g * 48:(t0g + blk) * 48])
            else:
                for g in range((eblk + 1023) // 1024):
                    nidx = min(1024, eblk - g * 1024)
                    i0 = e0 // 16 + g * 64
                    nc.gpsimd.dma_gather(
                        fe_sb[:, g * 8:g * 8 + nidx // 128, :], fpad_d[:],
                        uw_sb[:, i0:i0 + nidx // 16],
                        num_idxs=nidx, num_idxs_reg=nidx, elem_size=FPAD,
                    )

            # h.T = relu(W1.T @ efT + b1): [128h, eblk] fp16
            hT_sb = hpool.tile([HIDDEN, eblk], dt.float16, tag="hT")
            for q in range(eblk // 512 if "nomlp" not in ABL else 0):
                hT_ps = pps.tile([HIDDEN, 512], dt.float32, tag="hTps")
                nc.tensor.matmul(hT_ps[:], w1_sb[:],
                                 efT_sb[:, q * 512:(q + 1) * 512],
                                 start=True, stop=True)
                nc.scalar.activation(hT_sb[:, q * 512:(q + 1) * 512], hT_ps[:],
                                     mybir.ActivationFunctionType.Relu,
                                     bias=b1_sb[:], scale=1.0)

            out_sb = opool.tile([128, blk, 48], dt.float16, tag="outsb")

            for tq in range(blk // TB):
                # rw for TB tiles: PE matmul + b2 ones-matmul, ACT eviction
                rw_sb = wpool.tile([128, TB, RW], dt.float16, tag="rwsb")
                for u in range(TB):
                    t = tq * TB + u
                    rw_ps = ppr.tile([128, RW], dt.float32, tag="rwps")
                    hT_c = hT_sb[:, t * 128:(t + 1) * 128]
                    if "nomlp" not in ABL:
                        nc.tensor.matmul(rw_ps[:, 0:512], hT_c, w2_sb[:, 0:512],
                                         start=True, stop=False)
                        nc.tensor.matmul(rw_ps[:, 0:512], ones_sb[:],
                                         b2_sb[:, 0:512], start=False, stop=True)
                        nc.tensor.matmul(rw_ps[:, 512:RW], hT_c, w2_sb[:, 512:RW],
                                         start=True, stop=False)
                        nc.tensor.matmul(rw_ps[:, 512:RW], ones_sb[:],
                                         b2_sb[:, 512:RW], start=False, stop=True)
                    nc.scalar.activation(rw_sb[:, u, :], rw_ps[:],
                                         mybir.ActivationFunctionType.Copy,
                                         bias=0.0, scale=1.0)

                if "noc" in ABL:
                    continue
                # products1: P[p,t,(i r),d,m] = rw[p,t,(i r),m] * fe[p,t,d,m]
                rw_b = rw_sb[:].rearrange(
                    "p t (ir m) -> p t ir m", ir=48, m=M1
                ).unsqueeze(3).broadcast_to([128, TB, 48, D1, M1])
                fe_b = fe_sb[:, tq * TB:(tq + 1) * TB, 0:48].rearrange(
                    "p t (d m) -> p t d m", d=D1, m=M1
                ).unsqueeze(2).broadcast_to([128, TB, 48, D1, M1])
                P = wpool.tile([128, TB, 48, D1, M1], dt.float16, tag="P")
                eng["P"].tensor_tensor(P[:], rw_b, fe_b, mul)
                A = spool.tile([128, TB, 48, D1], dt.float16, tag="A")
                qidx = t0g // TB + tq
                use_pe = MSUM == "pe" or (MSUM == "mix" and
                                          qidx % MIXD < MIXN)
                if use_pe:
                    # m-sum on PE: 16 identity-weight matmuls accumulate the
                    # m-slices of P into PSUM (half-quad: N=288 <= 512 fp32)
                    for h in range(TB // 2):
                        A_ps = ppa.tile([128, 2, 48, D1], dt.float32, tag="Aps")
                        for j in range(M1):
                            nc.tensor.matmul(A_ps[:], ident_sb[:],
                                             P[:, h * 2:(h + 1) * 2, :, :, j],
                                             start=(j == 0), stop=(j == M1 - 1))
                        nc.scalar.activation(A[:, h * 2:(h + 1) * 2], A_ps[:],
                                             mybir.ActivationFunctionType.Copy,
                                             bias=0.0, scale=1.0)
                else:
                    # tree over m on DVE/gpsimd: 16 -> 8 -> 4 -> 2 -> 1
                    P8 = wpool.tile([128, TB, 48, D1, 8], dt.float16, tag="P8")
                    if L1V >= 48:
                        eng["L1"].tensor_tensor(P8[:], P[:, :, :, :, 0:8],
                                                P[:, :, :, :, 8:16], add)
                    else:
                        nc.vector.tensor_tensor(P8[:, :, 0:L1V], P[:, :, 0:L1V, :, 0:8],
                                                P[:, :, 0:L1V, :, 8:16], add)
                        nc.gpsimd.tensor_tensor(P8[:, :, L1V:48], P[:, :, L1V:48, :, 0:8],
                                                P[:, :, L1V:48, :, 8:16], add)
                    P4 = spool.tile([128, TB, 48, D1, 4], dt.float16, tag="P4")
                    eng["L2"].tensor_tensor(P4[:], P8[:, :, :, :, 0:4],
                                            P8[:, :, :, :, 4:8], add)
                    P2 = spool.tile([128, TB, 48, D1, 2], dt.float16, tag="P2")
                    eng["L3"].tensor_tensor(P2[:], P4[:, :, :, :, 0:2],
                                            P4[:, :, :, :, 2:4], add)
                    eng["L4"].tensor_tensor(A[:], P2[:, :, :, :, 0],
                                            P2[:, :, :, :, 1], add)

                # products2 per tile: G[p,i,dd,(r d)] = A * basis
                Gq = spool.tile([128, TB, 48, RD], dt.float16, tag="Gq")
                for u in range(TB):
                    t = tq * TB + u
                    A_b = A[:, u].rearrange(
                        "p (i r) d -> p i (r d)", i=M2, r=NREPS
                    ).unsqueeze(2).broadcast_to([128, M2, D2, RD])
                    ba_b = basis_sb[:, t].rearrange(
                        "p (dd rd) -> p dd rd", dd=D2, rd=RD
                    ).unsqueeze(1).broadcast_to([128, M2, D2, RD])
                    g_t = Gq[:, u].rearrange(
                        "p (i dd) rd -> p i dd rd", i=M2, dd=D2)
                    geng[u].tensor_tensor(g_t, A_b, ba_b, mul)
                # tree over (r d): 9 = 8 + 1
                T1 = spool.tile([128, TB, 48, 4], dt.float16, tag="T1")
                eng["T1"].tensor_tensor(T1[:], Gq[:, :, :, 0:4],
                                        Gq[:, :, :, 4:8], add)
                T2 = spool.tile([128, TB, 48, 2], dt.float16, tag="T2")
                eng["T2"].tensor_tensor(T2[:], T1[:, :, :, 0:2],
                                        T1[:, :, :, 2:4], add)
                T3 = spool.tile([128, TB, 48], dt.float16, tag="T3")
                eng["T3"].tensor_tensor(T3[:], T2[:, :, :, 0], T2[:, :, :, 1],
                                        add)
                eng["T4"].tensor_tensor(out_sb[:, tq * TB:(tq + 1) * TB, :],
                                        T3[:], Gq[:, :, :, 8], add)

            if "noc" not in ABL:
                nc.sync.dma_start(out_d[:, t0g * 48:(t0g + blk) * 48],
                                  out_sb[:].rearrange("p b k -> p (b k)"))
            t0g += blk


def _get_nc(reps=1):
    key = ("nc", reps)
    if key not in _CACHE:
        _CACHE[key] = _build(reps)
    return _CACHE[key]


def _prep_core(U_c, basis_c, ef_c, f, W1, b1, W2, b2):
    """Build one core's input map (host-side layout/swizzle)."""
    ne = U_c.shape[0]
    U_p = np.concatenate([np.asarray(U_c, np.int64),
                          np.zeros(ESP_G - ne, np.int64)])
    basis_p = np.concatenate(
        [np.asarray(basis_c, np.float32).reshape(-1, D1, NREPS * D2),
         np.zeros((ESP - ne, D1, NREPS * D2), np.float32)], axis=0)
    ef_p = np.concatenate(
        [np.asarray(ef_c, np.float32),
         np.zeros((ESP - ne, EDGE_DIM), np.float32)], axis=0)

    efT = np.ascontiguousarray(ef_p.T).astype(np.float16)                # [32, ESP]
    # basis edge layout (dd, r, d): idx = dd*9 + r*3 + d from in [d, r*3+dd]
    bp = basis_p.reshape(ESP, D1, NREPS, D2)          # [e, d, r, dd]
    bp = bp.transpose(0, 3, 2, 1)                     # [e, dd, r, d]
    bp = bp.reshape(ESP, 27)
    basisp = np.ascontiguousarray(
        bp.reshape(NTILES, 128, 27).transpose(1, 0, 2).reshape(128, NTILES * 27)
    ).astype(np.float16)
    # source-node features, d-major per node: [N, (d, m)]
    f48 = np.ascontiguousarray(
        np.asarray(f, np.float32).transpose(0, 2, 1).reshape(N, D1 * M1)
    ).astype(np.float16)
    if HOSTG:
        fe_all = f48[U_p[:ESP]]                                          # [ESP, 48]
        fmaps = {"fep": np.ascontiguousarray(
            fe_all.reshape(NTILES, 128, 48).transpose(1, 0, 2)
            .reshape(128, NTILES * 48))}
    else:
        uw16 = U_p.astype(np.int16).reshape(ESP_G // 1024, 64, 16).transpose(2, 0, 1)
        fpad = np.zeros((N, FPAD), np.float16)
        fpad[:, :M1 * D1] = f48
        fmaps = {
            "uw": np.ascontiguousarray(
                np.tile(uw16.reshape(16, ESP_G // 16), (8, 1))),
            "fpad": fpad,
        }
    # W2/b2 column reorder: (i, m, r) -> (i, r, m)
    w2r = np.asarray(W2, np.float32).reshape(HIDDEN, M2, M1, NREPS)
    w2r = np.ascontiguousarray(w2r.transpose(0, 1, 3, 2).reshape(HIDDEN, RW))
    b2r = np.asarray(b2, np.float32).reshape(M2, M1, NREPS)
    b2r = np.ascontiguousarray(b2r.transpose(0, 2, 1).reshape(1, RW))
    return {
        "efT": efT,
        "basisp": basisp,
        **fmaps,
        "w1": np.asarray(W1, np.float32).astype(np.float16),
        "b1": np.asarray(b1, np.float32).reshape(HIDDEN, 1),
        "w2r": w2r.astype(np.float16),
        "b2r": b2r.astype(np.float16),
        "ones1": np.ones((1, 128), np.float16),
        "ident": np.eye(128, dtype=np.float16),
    }


def kernel(U, basis, edge_feats, f, W1, b1, W2, b2):
    U = np.asarray(U)
    basis = np.asarray(basis, np.float32)
    edge_feats = np.asarray(edge_feats, np.float32)
    nc = _get_nc()
    in_maps = []
    for c in range(NCORES):
        sl = slice(c * ES, (c + 1) * ES)
        in_maps.append(_prep_core(U[sl], basis[sl], edge_feats[sl],
                                  f, W1, b1, W2, b2))
    res = run_bass_kernel_spmd(nc, in_maps, core_ids=list(range(NCORES)))
    outs = []
    for c in range(NCORES):
        op = res.results[c]["outp"]                                   # [128, NTILES*48] fp16
        o = op.astype(np.float32).reshape(128, NTILES, 48)
        o = o.transpose(1, 0, 2).reshape(ESP, 48)
        outs.append(o[:ES])
    return np.concatenate(outs, axis=0).reshape(E, M2, D2).astype(np.float32)


if __name__ == "__main__":
    rng = np.random.default_rng(0)
    inputs = {
        "U": rng.integers(0, N, size=E),
        "basis": rng.standard_normal((E, D1, NREPS * D2), dtype=np.float32),
        "edge_feats": rng.standard_normal((E, EDGE_DIM), dtype=np.float32),
        "f": rng.standard_normal((N, M1, D1), dtype=np.float32),
        "W1": (rng.standard_normal((EDGE_DIM, HIDDEN), dtype=np.float32) / np.sqrt(EDGE_DIM)),
        "b1": rng.standard_normal(HIDDEN, dtype=np.float32) * 0.02,
        "W2": (rng.standard_normal((HIDDEN, RW), dtype=np.float32) / np.sqrt(HIDDEN)),
        "b2": rng.standard_normal(RW, dtype=np.float32) * 0.02,
    }
    out = kernel(**inputs)
    print(out.shape, out.dtype)


# revision 38
# speedup vs baseline: 1.3126x; 1.0679x over previous
"""Trainium2 Bass kernel for nn_EquivariantConvolution (gnn_message_passing).

Math (per edge e):
    h  = relu(edge_feats @ W1 + b1)            [E,128]
    rw = (h @ W2 + b2) -> [E, 16, 48]
    fe = f[U]                                  [E,16,3]
    tmp[e,m,k] = sum_d fe[e,m,d] * basis[e,d,k]        (k = r*3+dd, 9)
    out[e,i,dd] = sum_{m,r} rw[e,i,m*3+r] * tmp[e,m,r*3+dd]

v2 "A-order" contraction (same result, fewer DVE passes):
    A[e,i,r,d]  = sum_m rw[e,(i,r,m)] * fe[e,(d,m)]     (contract m=16)
    out[e,i,dd] = sum_{r,d} A[e,i,(r,d)] * basis[e,(dd,r,d)]  (contract 9)

Sharding: edges split across 8 cores (40000 each, padded to 40960);
f + MLP weights replicated. Edge j of a 128-edge tile on partition j%128.

Engines: PE does the two MLP GEMMs (+b2 via ones-matmul into PSUM);
ACT evicts rw PSUM->SBUF fp16; gpsimd drives the f[U] dma_gather; DVE
does all per-edge contraction math in fp16 2x mode (products m-packed,
then a halving add-tree per contraction). Instructions are batched over
KTB=4 tiles to amortize fixed per-instruction costs.

HW-measured dead ends (sim underprices both): gpsimd tensor_tensor has
~1-2us fixed cost per instruction (Q7 software), and small-N
strided-ifmap PE matmuls (identity-weight segment sums) cost ~340ns
each vs 120ns modeled. Both offload paths remain behind KENG/KMSUM
env knobs but default off.
"""
import sys

sys.path.insert(0, "/opt/trn_rl_repo")

import os
import numpy as np
import concourse.bass as bass
import concourse.bacc as bacc
import concourse.mybir as mybir
import concourse.tile as tile
from concourse.bass_utils import run_bass_kernel_spmd
from contextlib import ExitStack

# problem constants (hardcoded per harness contract)
E = 320000
N = 10000
M1 = 16
M2 = 16
D1 = 3
D2 = 3
NREPS = 3
EDGE_DIM = 32
HIDDEN = 128
RW = NREPS * M1 * M2  # 768

NCORES = 8
ES = E // NCORES          # 40000 edges per core
ESP_G = 40960             # gather-side pad (40 groups of 1024 idxs)
NTILES = int(os.environ.get('KNT', '320'))     # compute tiles (all 1024-aligned)
ESP = NTILES * 128        # compute-side padded edges per core
BLK = int(os.environ.get('KBLK', '32'))        # tiles per (full) block
BLKS = [BLK] * (NTILES // BLK)                 # per-block tile counts
if NTILES % BLK:
    BLKS.append(NTILES % BLK)
TB = int(os.environ.get('KTB', '8'))           # tiles batched per DVE inst
assert all(b % TB == 0 and (b * 128) % 512 == 0 for b in BLKS)
TILES_REAL = (ES + 127) // 128                 # 313: tiles with real edges

_CACHE = {}

ABL = set(os.environ.get("KABL", "").split(","))  # ablation flags
HOSTG = os.environ.get("KHOSTG", "0") == "1"      # gather f[U] on host
FPAD = 128                # f rows padded to 128 fp16 (256B) for dma_gather

# per-op engine map: v = DVE (vector), g = gpsimd (Pool). For G (one inst
# per tile in the TB batch), a multi-char string assigns per tile index.
_ENG_DEFAULT = "P:v,L1:v,L2:v,L3:v,L4:v,G:vvvv,T1:v,T2:v,T3:v,T4:v"
ENG_MAP = dict(kv.split(":") for kv in
               os.environ.get("KENG", _ENG_DEFAULT).split(","))
L1V = int(os.environ.get("KL1V", "48"))  # ir columns of L1 on DVE; rest gpsimd
MSUM = os.environ.get("KMSUM", "dve")    # m-sum: pe | dve | mix (per-quad)
MIXN = int(os.environ.get("KMIXN", "1"))  # mix: PE-path quads per KMIXD
MIXD = int(os.environ.get("KMIXD", "2"))


def _build(reps=1):
    dt = mybir.dt
    nc = bacc.Bacc("TRN2", target_bir_lowering=False, debug=False,
                   num_devices=NCORES)

    efT_d = nc.dram_tensor("efT", [EDGE_DIM, ESP], dt.float16, kind="ExternalInput").ap()
    basis_d = nc.dram_tensor("basisp", [128, NTILES * 27], dt.float16, kind="ExternalInput").ap()
    if HOSTG:
        fep_d = nc.dram_tensor("fep", [128, NTILES * 48], dt.float16, kind="ExternalInput").ap()
    else:
        uw_d = nc.dram_tensor("uw", [128, ESP_G // 16], dt.int16, kind="ExternalInput").ap()
        fpad_d = nc.dram_tensor("fpad", [N, FPAD], dt.float16, kind="ExternalInput").ap()
        fep_d = None
    w1_d = nc.dram_tensor("w1", [EDGE_DIM, HIDDEN], dt.float16, kind="ExternalInput").ap()
    b1_d = nc.dram_tensor("b1", [HIDDEN, 1], dt.float32, kind="ExternalInput").ap()
    w2_d = nc.dram_tensor("w2r", [HIDDEN, RW], dt.float16, kind="ExternalInput").ap()
    b2_d = nc.dram_tensor("b2r", [1, RW], dt.float16, kind="ExternalInput").ap()
    ones_d = nc.dram_tensor("ones1", [1, 128], dt.float16, kind="ExternalInput").ap()
    ident_d = nc.dram_tensor("ident", [128, 128], dt.float16, kind="ExternalInput").ap()
    out_d = nc.dram_tensor("outp", [128, NTILES * 48], dt.float16, kind="ExternalOutput").ap()

    with tile.TileContext(nc) as tc, ExitStack() as ctx:
        cpool = ctx.enter_context(tc.tile_pool(name="const", bufs=1))
        inpool = ctx.enter_context(tc.tile_pool(name="in", bufs=2))
        hpool = ctx.enter_context(tc.tile_pool(name="h", bufs=2))
        wpool = ctx.enter_context(tc.tile_pool(name="work", bufs=2))
        # DVE-internal chain tiles: produced and consumed in program order
        # on the one vector engine, so a single buffer never stalls
        spool = ctx.enter_context(tc.tile_pool(name="small", bufs=1))
        opool = ctx.enter_context(tc.tile_pool(name="out", bufs=2))
        pps = ctx.enter_context(tc.tile_pool(name="psA", bufs=2, space="PSUM"))
        ppr = ctx.enter_context(tc.tile_pool(name="psB", bufs=2, space="PSUM"))
        ppa = ctx.enter_context(tc.tile_pool(name="psC", bufs=2, space="PSUM"))

        w1_sb = cpool.tile([EDGE_DIM, HIDDEN], dt.float16)
        nc.sync.dma_start(w1_sb[:], w1_d[:])
        b1_sb = cpool.tile([HIDDEN, 1], dt.float32)
        nc.sync.dma_start(b1_sb[:], b1_d[:])
        w2_sb = cpool.tile([HIDDEN, RW], dt.float16)
        nc.sync.dma_start(w2_sb[:], w2_d[:])
        b2_sb = cpool.tile([1, RW], dt.float16)
        nc.sync.dma_start(b2_sb[:], b2_d[:])
        ones_sb = cpool.tile([1, 128], dt.float16)
        nc.sync.dma_start(ones_sb[:], ones_d[:])
        ident_sb = cpool.tile([128, 128], dt.float16)
        nc.sync.dma_start(ident_sb[:], ident_d[:])
        if HOSTG:
            uw_sb = fpad_dd = None
        else:
            uw_sb = cpool.tile([128, ESP_G // 16], dt.int16)
            nc.sync.dma_start(uw_sb[:], uw_d[:])
            fpad_dd = fpad_d

        def body():
            _body(nc, tc, dt, inpool, hpool, wpool, spool, opool, pps, ppr,
                  ppa, efT_d, basis_d, fep_d, uw_sb, fpad_dd,
                  w1_sb, b1_sb, w2_sb, b2_sb, ones_sb, ident_sb, out_d)

        if reps == 1:
            body()
        else:
            with tc.For_i(0, reps, 1):
                body()

    nc.compile()
    return nc


def _body(nc, tc, dt, inpool, hpool, wpool, spool, opool, pps, ppr,
          ppa, efT_d, basis_d, fep_d, uw_sb, fpad_d,
          w1_sb, b1_sb, w2_sb, b2_sb, ones_sb, ident_sb, out_d):
    RD = NREPS * D1   # 9
    FW = 48 if HOSTG else FPAD
    mul = mybir.AluOpType.mult
    add = mybir.AluOpType.add
    def _e(ch):
        return nc.vector if ch == "v" else nc.gpsimd
    eng = {k: _e(v[0]) for k, v in ENG_MAP.items()}
    geng = [_e(ENG_MAP["G"][u % len(ENG_MAP["G"])]) for u in range(TB)]
    with nc.allow_low_precision(reason="fp16 pipeline; abs gate 2e-2"):
        t0g = 0  # first tile of this block
        for b, blk in enumerate(BLKS):
            eblk = blk * 128
            e0 = t0g * 128
            # block loads
            efT_sb = inpool.tile([EDGE_DIM, eblk], dt.float16, tag="efT")
            nc.sync.dma_start(efT_sb[:], efT_d[:, e0:e0 + eblk])
            basis_sb = inpool.tile([128, blk, 27], dt.float16, tag="basis")
            nc.sync.dma_start(basis_sb[:].rearrange("p b k -> p (b k)"),
                              basis_d[:, t0g * 27:(t0g + blk) * 27])
            fe_sb = inpool.tile([128, blk, FW], dt.float16, tag="fe")
            if HOSTG:
                nc.sync.dma_start(
                    fe_sb[:].rearrange("p b k -> p (b k)"),
                    fep_d[:, t0g * 48:(t0g + blk) * 48])
            else:
                for g in range((eblk + 1023) // 1024):
                    nidx = min(1024, eblk - g * 1024)
                    i0 = e0 // 16 + g * 64
                    nc.gpsimd.dma_gather(
                        fe_sb[:, g * 8:g * 8 + nidx // 128, :], fpad_d[:],
                        uw_sb[:, i0:i0 + nidx // 16],
                        num_idxs=nidx, num_idxs_reg=nidx, elem_size=FPAD,
                    )

            # h.T = relu(W1.T @ efT + b1): [128h, eblk] fp16
            hT_sb = hpool.tile([HIDDEN, eblk], dt.float16, tag="hT")
            for q in range(eblk // 512 if "nomlp" not in ABL else 0):
                hT_ps = pps.tile([HIDDEN, 512], dt.float32, tag="hTps")
                nc.tensor.matmul(hT_ps[:], w1_sb[:],
                                 efT_sb[:, q * 512:(q + 1) * 512],
                                 start=True, stop=True)
                nc.scalar.activation(hT_sb[:, q * 512:(q + 1) * 512], hT_ps[:],
                                     mybir.ActivationFunctionType.Relu,
                                     bias=b1_sb[:], scale=1.0)

            out_sb = opool.tile([128, blk, 48], dt.float16, tag="outsb")

            for tq in range(blk // TB):
                # tiles with real edges in this batch (skip pure padding)
                tb = min(TB, TILES_REAL - (t0g + tq * TB))
                if tb <= 0:
                    continue
                # rw for tb tiles: PE matmul + b2 ones-matmul, ACT eviction
                rw_sb = wpool.tile([128, tb, RW], dt.float16, tag="rwsb")
                for u in range(tb):
                    t = tq * TB + u
                    rw_ps = ppr.tile([128, RW], dt.float32, tag="rwps")
                    hT_c = hT_sb[:, t * 128:(t + 1) * 128]
                    if "nomlp" not in ABL:
                        nc.tensor.matmul(rw_ps[:, 0:512], hT_c, w2_sb[:, 0:512],
                                         start=True, stop=False)
                        nc.tensor.matmul(rw_ps[:, 0:512], ones_sb[:],
                                         b2_sb[:, 0:512], start=False, stop=True)
                        nc.tensor.matmul(rw_ps[:, 512:RW], hT_c, w2_sb[:, 512:RW],
                                         start=True, stop=False)
                        nc.tensor.matmul(rw_ps[:, 512:RW], ones_sb[:],
                                         b2_sb[:, 512:RW], start=False, stop=True)
                    nc.scalar.activation(rw_sb[:, u, :], rw_ps[:],
                                         mybir.ActivationFunctionType.Copy,
                                         bias=0.0, scale=1.0)

                if "noc" in ABL:
                    continue
                # products1: P[p,t,(i r),d,m] = rw[p,t,(i r),m] * fe[p,t,d,m]
                rw_b = rw_sb[:].rearrange(
                    "p t (ir m) -> p t ir m", ir=48, m=M1
                ).unsqueeze(3).broadcast_to([128, tb, 48, D1, M1])
                fe_b = fe_sb[:, tq * TB:tq * TB + tb, 0:48].rearrange(
                    "p t (d m) -> p t d m", d=D1, m=M1
                ).unsqueeze(2).broadcast_to([128, tb, 48, D1, M1])
                P = spool.tile([128, tb, 48, D1, M1], dt.float16, tag="P")
                eng["P"].tensor_tensor(P[:], rw_b, fe_b, mul)
                A = spool.tile([128, tb, 48, D1], dt.float16, tag="A")
                qidx = t0g // TB + tq
                use_pe = (MSUM == "pe" or (MSUM == "mix" and
                                           qidx % MIXD < MIXN)) and tb == TB
                if use_pe:
                    # m-sum on PE: 16 identity-weight matmuls accumulate the
                    # m-slices of P into PSUM (half-quad: N=288 <= 512 fp32)
                    for h in range(TB // 2):
                        A_ps = ppa.tile([128, 2, 48, D1], dt.float32, tag="Aps")
                        for j in range(M1):
                            nc.tensor.matmul(A_ps[:], ident_sb[:],
                                             P[:, h * 2:(h + 1) * 2, :, :, j],
                                             start=(j == 0), stop=(j == M1 - 1))
                        nc.scalar.activation(A[:, h * 2:(h + 1) * 2], A_ps[:],
                                             mybir.ActivationFunctionType.Copy,
                                             bias=0.0, scale=1.0)
                else:
                    # tree over m on DVE/gpsimd: 16 -> 8 -> 4 -> 2 -> 1
                    P8 = spool.tile([128, tb, 48, D1, 8], dt.float16, tag="P8")
                    if L1V >= 48:
                        eng["L1"].tensor_tensor(P8[:], P[:, :, :, :, 0:8],
                                                P[:, :, :, :, 8:16], add)
                    else:
                        nc.vector.tensor_tensor(P8[:, :, 0:L1V], P[:, :, 0:L1V, :, 0:8],
                                                P[:, :, 0:L1V, :, 8:16], add)
                        nc.gpsimd.tensor_tensor(P8[:, :, L1V:48], P[:, :, L1V:48, :, 0:8],
                                                P[:, :, L1V:48, :, 8:16], add)
                    P4 = spool.tile([128, tb, 48, D1, 4], dt.float16, tag="P4")
                    eng["L2"].tensor_tensor(P4[:], P8[:, :, :, :, 0:4],
                                            P8[:, :, :, :, 4:8], add)
                    P2 = spool.tile([128, tb, 48, D1, 2], dt.float16, tag="P2")
                    eng["L3"].tensor_tensor(P2[:], P4[:, :, :, :, 0:2],
                                            P4[:, :, :, :, 2:4], add)
                    eng["L4"].tensor_tensor(A[:], P2[:, :, :, :, 0],
                                            P2[:, :, :, :, 1], add)

                # products2 per tile: G[p,i,dd,(r d)] = A * basis
                Gq = spool.tile([128, tb, 48, RD], dt.float16, tag="Gq")
                for u in range(tb):
                    t = tq * TB + u
                    A_b = A[:, u].rearrange(
                        "p (i r) d -> p i (r d)", i=M2, r=NREPS
                    ).unsqueeze(2).broadcast_to([128, M2, D2, RD])
                    ba_b = basis_sb[:, t].rearrange(
                        "p (dd rd) -> p dd rd", dd=D2, rd=RD
                    ).unsqueeze(1).broadcast_to([128, M2, D2, RD])
                    g_t = Gq[:, u].rearrange(
                        "p (i dd) rd -> p i dd rd", i=M2, dd=D2)
                    geng[u].tensor_tensor(g_t, A_b, ba_b, mul)
                # tree over (r d): 9 = 8 + 1
                T1 = spool.tile([128, tb, 48, 4], dt.float16, tag="T1")
                eng["T1"].tensor_tensor(T1[:], Gq[:, :, :, 0:4],
                                        Gq[:, :, :, 4:8], add)
                T2 = spool.tile([128, tb, 48, 2], dt.float16, tag="T2")
                eng["T2"].tensor_tensor(T2[:], T1[:, :, :, 0:2],
                                        T1[:, :, :, 2:4], add)
                T3 = spool.tile([128, tb, 48], dt.float16, tag="T3")
                eng["T3"].tensor_tensor(T3[:], T2[:, :, :, 0], T2[:, :, :, 1],
                                        add)
                eng["T4"].tensor_tensor(out_sb[:, tq * TB:tq * TB + tb, :],
                                        T3[:], Gq[:, :, :, 8], add)

            wblk = min(blk, TILES_REAL - t0g)   # real tiles to write back
            if "noc" not in ABL and wblk > 0:
                nc.sync.dma_start(out_d[:, t0g * 48:(t0g + wblk) * 48],
                                  out_sb[:, 0:wblk, :].rearrange(
                                      "p b k -> p (b k)"))
            t0g += blk


def _get_nc(reps=1):
    key = ("nc", reps)
    if key not in _CACHE:
        _CACHE[key] = _build(reps)
    return _CACHE[key]


def _prep_core(U_c, basis_c, ef_c, f, W1, b1, W2, b2):
    """Build one core's input map (host-side layout/swizzle)."""
    ne = U_c.shape[0]
    U_p = np.concatenate([np.asarray(U_c, np.int64),
                          np.zeros(ESP_G - ne, np.int64)])
    basis_p = np.concatenate(
        [np.asarray(basis_c, np.float32).reshape(-1, D1, NREPS * D2),
         np.zeros((ESP - ne, D1, NREPS * D2), np.float32)], axis=0)
    ef_p = np.concatenate(
        [np.asarray(ef_c, np.float32),
         np.zeros((ESP - ne, EDGE_DIM), np.float32)], axis=0)

    efT = np.ascontiguousarray(ef_p.T).astype(np.float16)                # [32, ESP]
    # basis edge layout (dd, r, d): idx = dd*9 + r*3 + d from in [d, r*3+dd]
    bp = basis_p.reshape(ESP, D1, NREPS, D2)          # [e, d, r, dd]
    bp = bp.transpose(0, 3, 2, 1)                     # [e, dd, r, d]
    bp = bp.reshape(ESP, 27)
    basisp = np.ascontiguousarray(
        bp.reshape(NTILES, 128, 27).transpose(1, 0, 2).reshape(128, NTILES * 27)
    ).astype(np.float16)
    # source-node features, d-major per node: [N, (d, m)]
    f48 = np.ascontiguousarray(
        np.asarray(f, np.float32).transpose(0, 2, 1).reshape(N, D1 * M1)
    ).astype(np.float16)
    if HOSTG:
        fe_all = f48[U_p[:ESP]]                                          # [ESP, 48]
        fmaps = {"fep": np.ascontiguousarray(
            fe_all.reshape(NTILES, 128, 48).transpose(1, 0, 2)
            .reshape(128, NTILES * 48))}
    else:
        uw16 = U_p.astype(np.int16).reshape(ESP_G // 1024, 64, 16).transpose(2, 0, 1)
        fpad = np.zeros((N, FPAD), np.float16)
        fpad[:, :M1 * D1] = f48
        fmaps = {
            "uw": np.ascontiguousarray(
                np.tile(uw16.reshape(16, ESP_G // 16), (8, 1))),
            "fpad": fpad,
        }
    # W2/b2 column reorder: (i, m, r) -> (i, r, m)
    w2r = np.asarray(W2, np.float32).reshape(HIDDEN, M2, M1, NREPS)
    w2r = np.ascontiguousarray(w2r.transpose(0, 1, 3, 2).reshape(HIDDEN, RW))
    b2r = np.asarray(b2, np.float32).reshape(M2, M1, NREPS)
    b2r = np.ascontiguousarray(b2r.transpose(0, 2, 1).reshape(1, RW))
    return {
        "efT": efT,
        "basisp": basisp,
        **fmaps,
        "w1": np.asarray(W1, np.float32).astype(np.float16),
        "b1": np.asarray(b1, np.float32).reshape(HIDDEN, 1),
        "w2r": w2r.astype(np.float16),
        "b2r": b2r.astype(np.float16),
        "ones1": np.ones((1, 128), np.float16),
        "ident": np.eye(128, dtype=np.float16),
    }


def kernel(U, basis, edge_feats, f, W1, b1, W2, b2):
    U = np.asarray(U)
    basis = np.asarray(basis, np.float32)
    edge_feats = np.asarray(edge_feats, np.float32)
    nc = _get_nc()
    in_maps = []
    for c in range(NCORES):
        sl = slice(c * ES, (c + 1) * ES)
        in_maps.append(_prep_core(U[sl], basis[sl], edge_feats[sl],
                                  f, W1, b1, W2, b2))
    res = run_bass_kernel_spmd(nc, in_maps, core_ids=list(range(NCORES)))
    outs = []
    for c in range(NCORES):
        op = res.results[c]["outp"]                                   # [128, NTILES*48] fp16
        o = op.astype(np.float32).reshape(128, NTILES, 48)
        o = o.transpose(1, 0, 2).reshape(ESP, 48)
        outs.append(o[:ES])
    return np.concatenate(outs, axis=0).reshape(E, M2, D2).astype(np.float32)


if __name__ == "__main__":
    rng = np.random.default_rng(0)
    inputs = {
        "U": rng.integers(0, N, size=E),
        "basis": rng.standard_normal((E, D1, NREPS * D2), dtype=np.float32),
        "edge_feats": rng.standard_normal((E, EDGE_DIM), dtype=np.float32),
        "f": rng.standard_normal((N, M1, D1), dtype=np.float32),
        "W1": (rng.standard_normal((EDGE_DIM, HIDDEN), dtype=np.float32) / np.sqrt(EDGE_DIM)),
        "b1": rng.standard_normal(HIDDEN, dtype=np.float32) * 0.02,
        "W2": (rng.standard_normal((HIDDEN, RW), dtype=np.float32) / np.sqrt(HIDDEN)),
        "b2": rng.standard_normal(RW, dtype=np.float32) * 0.02,
    }
    out = kernel(**inputs)
    print(out.shape, out.dtype)


# revision 39
# speedup vs baseline: 1.4292x; 1.0888x over previous
"""Trainium2 Bass kernel for nn_EquivariantConvolution (gnn_message_passing).

Math (per edge e):
    h  = relu(edge_feats @ W1 + b1)            [E,128]
    rw = (h @ W2 + b2) -> [E, 16, 48]
    fe = f[U]                                  [E,16,3]
    tmp[e,m,k] = sum_d fe[e,m,d] * basis[e,d,k]        (k = r*3+dd, 9)
    out[e,i,dd] = sum_{m,r} rw[e,i,m*3+r] * tmp[e,m,r*3+dd]

v2 "A-order" contraction (same result, fewer DVE passes):
    A[e,i,r,d]  = sum_m rw[e,(i,r,m)] * fe[e,(d,m)]     (contract m=16)
    out[e,i,dd] = sum_{r,d} A[e,i,(r,d)] * basis[e,(dd,r,d)]  (contract 9)

Sharding: edges split across 8 cores (40000 each, padded to 40960);
f + MLP weights replicated. Edge j of a 128-edge tile on partition j%128.

Engines: PE does the two MLP GEMMs (+b2 via ones-matmul into PSUM);
ACT evicts rw PSUM->SBUF fp16; gpsimd drives the f[U] dma_gather; DVE
does all per-edge contraction math in fp16 2x mode (products m-packed,
then a halving add-tree per contraction). Instructions are batched over
KTB=8 tiles to amortize fixed per-instruction costs (DVE-internal chain
tiles are single-buffered: in-order execution makes WAR reuse free),
and compute is skipped for the 7 pure-padding tiles past tile 313.

HW-measured dead ends (sim underprices both): gpsimd tensor_tensor has
~1-2us fixed cost per instruction (Q7 software), and small-N
strided-ifmap PE matmuls (identity-weight segment sums) cost ~340ns
each vs 120ns modeled. Both offload paths remain behind KENG/KMSUM
env knobs but default off.
"""
import sys

sys.path.insert(0, "/opt/trn_rl_repo")

import os
import numpy as np
import concourse.bass as bass
import concourse.bacc as bacc
import concourse.mybir as mybir
import concourse.tile as tile
from concourse.bass_utils import run_bass_kernel_spmd
from contextlib import ExitStack

# problem constants (hardcoded per harness contract)
E = 320000
N = 10000
M1 = 16
M2 = 16
D1 = 3
D2 = 3
NREPS = 3
EDGE_DIM = 32
HIDDEN = 128
RW = NREPS * M1 * M2  # 768

NCORES = 8
ES = E // NCORES          # 40000 edges per core
ESP_G = 40960             # gather-side pad (40 groups of 1024 idxs)
NTILES = int(os.environ.get('KNT', '320'))     # compute tiles (all 1024-aligned)
ESP = NTILES * 128        # compute-side padded edges per core
BLK = int(os.environ.get('KBLK', '32'))        # tiles per (full) block
BLKS = [BLK] * (NTILES // BLK)                 # per-block tile counts
if NTILES % BLK:
    BLKS.append(NTILES % BLK)
TB = int(os.environ.get('KTB', '8'))           # tiles batched per DVE inst
assert all(b % TB == 0 and (b * 128) % 512 == 0 for b in BLKS)
TILES_REAL = (ES + 127) // 128                 # 313: tiles with real edges

_CACHE = {}

ABL = set(os.environ.get("KABL", "").split(","))  # ablation flags
HOSTG = os.environ.get("KHOSTG", "0") == "1"      # gather f[U] on host
FPAD = 128                # f rows padded to 128 fp16 (256B) for dma_gather

# per-op engine map: v = DVE (vector), g = gpsimd (Pool). For G (one inst
# per tile in the TB batch), a multi-char string assigns per tile index.
_ENG_DEFAULT = "P:v,L1:v,L2:v,L3:v,L4:v,G:vvvv,T1:v,T2:v,T3:v,T4:v"
ENG_MAP = dict(kv.split(":") for kv in
               os.environ.get("KENG", _ENG_DEFAULT).split(","))
L1V = int(os.environ.get("KL1V", "48"))  # ir columns of L1 on DVE; rest gpsimd
MSUM = os.environ.get("KMSUM", "dve")    # m-sum: pe | dve | mix (per-quad)
MIXN = int(os.environ.get("KMIXN", "1"))  # mix: PE-path quads per KMIXD
MIXD = int(os.environ.get("KMIXD", "2"))


def _build(reps=1):
    dt = mybir.dt
    nc = bacc.Bacc("TRN2", target_bir_lowering=False, debug=False,
                   num_devices=NCORES)

    efT_d = nc.dram_tensor("efT", [EDGE_DIM, ESP], dt.float16, kind="ExternalInput").ap()
    basis_d = nc.dram_tensor("basisp", [128, NTILES * 27], dt.float16, kind="ExternalInput").ap()
    if HOSTG:
        fep_d = nc.dram_tensor("fep", [128, NTILES * 48], dt.float16, kind="ExternalInput").ap()
    else:
        uw_d = nc.dram_tensor("uw", [128, ESP_G // 16], dt.int16, kind="ExternalInput").ap()
        fpad_d = nc.dram_tensor("fpad", [N, FPAD], dt.float16, kind="ExternalInput").ap()
        fep_d = None
    w1_d = nc.dram_tensor("w1", [EDGE_DIM, HIDDEN], dt.float16, kind="ExternalInput").ap()
    b1_d = nc.dram_tensor("b1", [HIDDEN, 1], dt.float32, kind="ExternalInput").ap()
    w2_d = nc.dram_tensor("w2r", [HIDDEN, RW], dt.float16, kind="ExternalInput").ap()
    b2_d = nc.dram_tensor("b2r", [1, RW], dt.float16, kind="ExternalInput").ap()
    ones_d = nc.dram_tensor("ones1", [1, 128], dt.float16, kind="ExternalInput").ap()
    ident_d = nc.dram_tensor("ident", [128, 128], dt.float16, kind="ExternalInput").ap()
    out_d = nc.dram_tensor("outp", [128, NTILES * 48], dt.float16, kind="ExternalOutput").ap()

    with tile.TileContext(nc) as tc, ExitStack() as ctx:
        cpool = ctx.enter_context(tc.tile_pool(name="const", bufs=1))
        inpool = ctx.enter_context(tc.tile_pool(name="in", bufs=2))
        hpool = ctx.enter_context(tc.tile_pool(name="h", bufs=2))
        wpool = ctx.enter_context(tc.tile_pool(name="work", bufs=2))
        # DVE-internal chain tiles: produced and consumed in program order
        # on the one vector engine, so a single buffer never stalls
        spool = ctx.enter_context(tc.tile_pool(name="small", bufs=1))
        opool = ctx.enter_context(tc.tile_pool(name="out", bufs=2))
        pps = ctx.enter_context(tc.tile_pool(name="psA", bufs=2, space="PSUM"))
        ppr = ctx.enter_context(tc.tile_pool(name="psB", bufs=2, space="PSUM"))
        ppa = ctx.enter_context(tc.tile_pool(name="psC", bufs=2, space="PSUM"))

        w1_sb = cpool.tile([EDGE_DIM, HIDDEN], dt.float16)
        nc.sync.dma_start(w1_sb[:], w1_d[:])
        b1_sb = cpool.tile([HIDDEN, 1], dt.float32)
        nc.sync.dma_start(b1_sb[:], b1_d[:])
        w2_sb = cpool.tile([HIDDEN, RW], dt.float16)
        nc.sync.dma_start(w2_sb[:], w2_d[:])
        b2_sb = cpool.tile([1, RW], dt.float16)
        nc.sync.dma_start(b2_sb[:], b2_d[:])
        ones_sb = cpool.tile([1, 128], dt.float16)
        nc.sync.dma_start(ones_sb[:], ones_d[:])
        ident_sb = cpool.tile([128, 128], dt.float16)
        nc.sync.dma_start(ident_sb[:], ident_d[:])
        if HOSTG:
            uw_sb = fpad_dd = None
        else:
            uw_sb = cpool.tile([128, ESP_G // 16], dt.int16)
            nc.sync.dma_start(uw_sb[:], uw_d[:])
            fpad_dd = fpad_d

        def body():
            _body(nc, tc, dt, inpool, hpool, wpool, spool, opool, pps, ppr,
                  ppa, efT_d, basis_d, fep_d, uw_sb, fpad_dd,
                  w1_sb, b1_sb, w2_sb, b2_sb, ones_sb, ident_sb, out_d)

        if reps == 1:
            body()
        else:
            with tc.For_i(0, reps, 1):
                body()

    nc.compile()
    return nc


def _body(nc, tc, dt, inpool, hpool, wpool, spool, opool, pps, ppr,
          ppa, efT_d, basis_d, fep_d, uw_sb, fpad_d,
          w1_sb, b1_sb, w2_sb, b2_sb, ones_sb, ident_sb, out_d):
    RD = NREPS * D1   # 9
    FW = 48 if HOSTG else FPAD
    mul = mybir.AluOpType.mult
    add = mybir.AluOpType.add
    def _e(ch):
        return nc.vector if ch == "v" else nc.gpsimd
    eng = {k: _e(v[0]) for k, v in ENG_MAP.items()}
    geng = [_e(ENG_MAP["G"][u % len(ENG_MAP["G"])]) for u in range(TB)]
    with nc.allow_low_precision(reason="fp16 pipeline; abs gate 2e-2"):
        t0g = 0  # first tile of this block
        for b, blk in enumerate(BLKS):
            eblk = blk * 128
            e0 = t0g * 128
            # block loads
            efT_sb = inpool.tile([EDGE_DIM, eblk], dt.float16, tag="efT")
            nc.sync.dma_start(efT_sb[:], efT_d[:, e0:e0 + eblk])
            basis_sb = inpool.tile([128, blk, 27], dt.float16, tag="basis")
            nc.sync.dma_start(basis_sb[:].rearrange("p b k -> p (b k)"),
                              basis_d[:, t0g * 27:(t0g + blk) * 27])
            fe_sb = inpool.tile([128, blk, FW], dt.float16, tag="fe")
            if HOSTG:
                nc.sync.dma_start(
                    fe_sb[:].rearrange("p b k -> p (b k)"),
                    fep_d[:, t0g * 48:(t0g + blk) * 48])
            else:
                for g in range((eblk + 1023) // 1024):
                    nidx = min(1024, eblk - g * 1024)
                    i0 = e0 // 16 + g * 64
                    nc.gpsimd.dma_gather(
                        fe_sb[:, g * 8:g * 8 + nidx // 128, :], fpad_d[:],
                        uw_sb[:, i0:i0 + nidx // 16],
                        num_idxs=nidx, num_idxs_reg=nidx, elem_size=FPAD,
                    )

            # h.T = relu(W1.T @ efT + b1): [128h, eblk] fp16
            hT_sb = hpool.tile([HIDDEN, eblk], dt.float16, tag="hT")
            for q in range(eblk // 512 if "nomlp" not in ABL else 0):
                hT_ps = pps.tile([HIDDEN, 512], dt.float32, tag="hTps")
                nc.tensor.matmul(hT_ps[:], w1_sb[:],
                                 efT_sb[:, q * 512:(q + 1) * 512],
                                 start=True, stop=True)
                nc.scalar.activation(hT_sb[:, q * 512:(q + 1) * 512], hT_ps[:],
                                     mybir.ActivationFunctionType.Relu,
                                     bias=b1_sb[:], scale=1.0)

            out_sb = opool.tile([128, blk, 48], dt.float16, tag="outsb")

            for tq in range(blk // TB):
                # tiles with real edges in this batch (skip pure padding)
                tb = min(TB, TILES_REAL - (t0g + tq * TB))
                if tb <= 0:
                    continue
                # rw for tb tiles: PE matmul + b2 ones-matmul, ACT eviction
                rw_sb = wpool.tile([128, tb, RW], dt.float16, tag="rwsb")
                for u in range(tb):
                    t = tq * TB + u
                    rw_ps = ppr.tile([128, RW], dt.float32, tag="rwps")
                    hT_c = hT_sb[:, t * 128:(t + 1) * 128]
                    if "nomlp" not in ABL:
                        nc.tensor.matmul(rw_ps[:, 0:512], hT_c, w2_sb[:, 0:512],
                                         start=True, stop=False)
                        nc.tensor.matmul(rw_ps[:, 0:512], ones_sb[:],
                                         b2_sb[:, 0:512], start=False, stop=True)
                        nc.tensor.matmul(rw_ps[:, 512:RW], hT_c, w2_sb[:, 512:RW],
                                         start=True, stop=False)
                        nc.tensor.matmul(rw_ps[:, 512:RW], ones_sb[:],
                                         b2_sb[:, 512:RW], start=False, stop=True)
                    nc.scalar.activation(rw_sb[:, u, :], rw_ps[:],
                                         mybir.ActivationFunctionType.Copy,
                                         bias=0.0, scale=1.0)

                if "noc" in ABL:
                    continue
                # products1: P[p,t,(i r),d,m] = rw[p,t,(i r),m] * fe[p,t,d,m]
                rw_b = rw_sb[:].rearrange(
                    "p t (ir m) -> p t ir m", ir=48, m=M1
                ).unsqueeze(3).broadcast_to([128, tb, 48, D1, M1])
                fe_b = fe_sb[:, tq * TB:tq * TB + tb, 0:48].rearrange(
                    "p t (d m) -> p t d m", d=D1, m=M1
                ).unsqueeze(2).broadcast_to([128, tb, 48, D1, M1])
                P = spool.tile([128, tb, 48, D1, M1], dt.float16, tag="P")
                eng["P"].tensor_tensor(P[:], rw_b, fe_b, mul)
                A = spool.tile([128, tb, 48, D1], dt.float16, tag="A")
                qidx = t0g // TB + tq
                use_pe = (MSUM == "pe" or (MSUM == "mix" and
                                           qidx % MIXD < MIXN)) and tb == TB
                if use_pe:
                    # m-sum on PE: 16 identity-weight matmuls accumulate the
                    # m-slices of P into PSUM (half-quad: N=288 <= 512 fp32)
                    for h in range(TB // 2):
                        A_ps = ppa.tile([128, 2, 48, D1], dt.float32, tag="Aps")
                        for j in range(M1):
                            nc.tensor.matmul(A_ps[:], ident_sb[:],
                                             P[:, h * 2:(h + 1) * 2, :, :, j],
                                             start=(j == 0), stop=(j == M1 - 1))
                        nc.scalar.activation(A[:, h * 2:(h + 1) * 2], A_ps[:],
                                             mybir.ActivationFunctionType.Copy,
                                             bias=0.0, scale=1.0)
                else:
                    # tree over m on DVE/gpsimd: 16 -> 8 -> 4 -> 2 -> 1
                    P8 = spool.tile([128, tb, 48, D1, 8], dt.float16, tag="P8")
                    if L1V >= 48:
                        eng["L1"].tensor_tensor(P8[:], P[:, :, :, :, 0:8],
                                                P[:, :, :, :, 8:16], add)
                    else:
                        nc.vector.tensor_tensor(P8[:, :, 0:L1V], P[:, :, 0:L1V, :, 0:8],
                                                P[:, :, 0:L1V, :, 8:16], add)
                        nc.gpsimd.tensor_tensor(P8[:, :, L1V:48], P[:, :, L1V:48, :, 0:8],
                                                P[:, :, L1V:48, :, 8:16], add)
                    P4 = spool.tile([128, tb, 48, D1, 4], dt.float16, tag="P4")
                    eng["L2"].tensor_tensor(P4[:], P8[:, :, :, :, 0:4],
                                            P8[:, :, :, :, 4:8], add)
                    P2 = spool.tile([128, tb, 48, D1, 2], dt.float16, tag="P2")
                    eng["L3"].tensor_tensor(P2[:], P4[:, :, :, :, 0:2],
                                            P4[:, :, :, :, 2:4], add)
                    eng["L4"].tensor_tensor(A[:], P2[:, :, :, :, 0],
                                            P2[:, :, :, :, 1], add)

                # products2 per tile: G[p,i,dd,(r d)] = A * basis
                Gq = spool.tile([128, tb, 48, RD], dt.float16, tag="Gq")
                for u in range(tb):
                    t = tq * TB + u
                    A_b = A[:, u].rearrange(
                        "p (i r) d -> p i (r d)", i=M2, r=NREPS
                    ).unsqueeze(2).broadcast_to([128, M2, D2, RD])
                    ba_b = basis_sb[:, t].rearrange(
                        "p (dd rd) -> p dd rd", dd=D2, rd=RD
                    ).unsqueeze(1).broadcast_to([128, M2, D2, RD])
                    g_t = Gq[:, u].rearrange(
                        "p (i dd) rd -> p i dd rd", i=M2, dd=D2)
                    geng[u].tensor_tensor(g_t, A_b, ba_b, mul)
                # tree over (r d): 9 = 8 + 1
                T1 = spool.tile([128, tb, 48, 4], dt.float16, tag="T1")
                eng["T1"].tensor_tensor(T1[:], Gq[:, :, :, 0:4],
                                        Gq[:, :, :, 4:8], add)
                T2 = spool.tile([128, tb, 48, 2], dt.float16, tag="T2")
                eng["T2"].tensor_tensor(T2[:], T1[:, :, :, 0:2],
                                        T1[:, :, :, 2:4], add)
                T3 = spool.tile([128, tb, 48], dt.float16, tag="T3")
                eng["T3"].tensor_tensor(T3[:], T2[:, :, :, 0], T2[:, :, :, 1],
                                        add)
                eng["T4"].tensor_tensor(out_sb[:, tq * TB:tq * TB + tb, :],
                                        T3[:], Gq[:, :, :, 8], add)

            wblk = min(blk, TILES_REAL - t0g)   # real tiles to write back
            if "noc" not in ABL and wblk > 0:
                nc.sync.dma_start(out_d[:, t0g * 48:(t0g + wblk) * 48],
                                  out_sb[:, 0:wblk, :].rearrange(
                                      "p b k -> p (b k)"))
            t0g += blk


def _get_nc(reps=1):
    key = ("nc", reps)
    if key not in _CACHE:
        _CACHE[key] = _build(reps)
    return _CACHE[key]


def _prep_core(U_c, basis_c, ef_c, f, W1, b1, W2, b2):
    """Build one core's input map (host-side layout/swizzle)."""
    ne = U_c.shape[0]
    U_p = np.concatenate([np.asarray(U_c, np.int64),
                          np.zeros(ESP_G - ne, np.int64)])
    basis_p = np.concatenate(
        [np.asarray(basis_c, np.float32).reshape(-1, D1, NREPS * D2),
         np.zeros((ESP - ne, D1, NREPS * D2), np.float32)], axis=0)
    ef_p = np.concatenate(
        [np.asarray(ef_c, np.float32),
         np.zeros((ESP - ne, EDGE_DIM), np.float32)], axis=0)

    efT = np.ascontiguousarray(ef_p.T).astype(np.float16)                # [32, ESP]
    # basis edge layout (dd, r, d): idx = dd*9 + r*3 + d from in [d, r*3+dd]
    bp = basis_p.reshape(ESP, D1, NREPS, D2)          # [e, d, r, dd]
    bp = bp.transpose(0, 3, 2, 1)                     # [e, dd, r, d]
    bp = bp.reshape(ESP, 27)
    basisp = np.ascontiguousarray(
        bp.reshape(NTILES, 128, 27).transpose(1, 0, 2).reshape(128, NTILES * 27)
    ).astype(np.float16)
    # source-node features, d-major per node: [N, (d, m)]
    f48 = np.ascontiguousarray(
        np.asarray(f, np.float32).transpose(0, 2, 1).reshape(N, D1 * M1)
    ).astype(np.float16)
    if HOSTG:
        fe_all = f48[U_p[:ESP]]                                          # [ESP, 48]
        fmaps = {"fep": np.ascontiguousarray(
            fe_all.reshape(NTILES, 128, 48).transpose(1, 0, 2)
            .reshape(128, NTILES * 48))}
    else:
        uw16 = U_p.astype(np.int16).reshape(ESP_G // 1024, 64, 16).transpose(2, 0, 1)
        fpad = np.zeros((N, FPAD), np.float16)
        fpad[:, :M1 * D1] = f48
        fmaps = {
            "uw": np.ascontiguousarray(
                np.tile(uw16.reshape(16, ESP_G // 16), (8, 1))),
            "fpad": fpad,
        }
    # W2/b2 column reorder: (i, m, r) -> (i, r, m)
    w2r = np.asarray(W2, np.float32).reshape(HIDDEN, M2, M1, NREPS)
    w2r = np.ascontiguousarray(w2r.transpose(0, 1, 3, 2).reshape(HIDDEN, RW))
    b2r = np.asarray(b2, np.float32).reshape(M2, M1, NREPS)
    b2r = np.ascontiguousarray(b2r.transpose(0, 2, 1).reshape(1, RW))
    return {
        "efT": efT,
        "basisp": basisp,
        **fmaps,
        "w1": np.asarray(W1, np.float32).astype(np.float16),
        "b1": np.asarray(b1, np.float32).reshape(HIDDEN, 1),
        "w2r": w2r.astype(np.float16),
        "b2r": b2r.astype(np.float16),
        "ones1": np.ones((1, 128), np.float16),
        "ident": np.eye(128, dtype=np.float16),
    }


def kernel(U, basis, edge_feats, f, W1, b1, W2, b2):
    U = np.asarray(U)
    basis = np.asarray(basis, np.float32)
    edge_feats = np.asarray(edge_feats, np.float32)
    nc = _get_nc()
    in_maps = []
    for c in range(NCORES):
        sl = slice(c * ES, (c + 1) * ES)
        in_maps.append(_prep_core(U[sl], basis[sl], edge_feats[sl],
                                  f, W1, b1, W2, b2))
    res = run_bass_kernel_spmd(nc, in_maps, core_ids=list(range(NCORES)))
    outs = []
    for c in range(NCORES):
        op = res.results[c]["outp"]                                   # [128, NTILES*48] fp16
        o = op.astype(np.float32).reshape(128, NTILES, 48)
        o = o.transpose(1, 0, 2).reshape(ESP, 48)
        outs.append(o[:ES])
    return np.concatenate(outs, axis=0).reshape(E, M2, D2).astype(np.float32)


if __name__ == "__main__":
    rng = np.random.default_rng(0)
    inputs = {
        "U": rng.integers(0, N, size=E),
        "basis": rng.standard_normal((E, D1, NREPS * D2), dtype=np.float32),
        "edge_feats": rng.standard_normal((E, EDGE_DIM), dtype=np.float32),
        "f": rng.standard_normal((N, M1, D1), dtype=np.float32),
        "W1": (rng.standard_normal((EDGE_DIM, HIDDEN), dtype=np.float32) / np.sqrt(EDGE_DIM)),
        "b1": rng.standard_normal(HIDDEN, dtype=np.float32) * 0.02,
        "W2": (rng.standard_normal((HIDDEN, RW), dtype=np.float32) / np.sqrt(HIDDEN)),
        "b2": rng.standard_normal(RW, dtype=np.float32) * 0.02,
    }
    out = kernel(**inputs)
    print(out.shape, out.dtype)
